# revision 9
# baseline (speedup 1.0000x reference)
"""CapsNet forward on 8 Trainium2 NeuronCores (Bass/Tile).

Data-parallel over batch B=180 (23/23/23/23/22/22/22/22 + pad-to-23 with a
duplicated masked image on the last 4 cores). Weights replicated. The only
cross-core communication: AllReduce of the [1152,10] routing agreement in
iterations 1 and 2 (iteration 3's update is dead in the reference).

Per-core compute (b = 23):
  conv1:  im2col x -> [81, b*400] fp32r; h = W1^T @ im2col -> [2][128, b*400]
  caps:   162 accumulating shift-matmuls (81 offsets x 2 in-chunks, fp32r),
          psum columns ordered (oy, ox, b) so b is innermost everywhere
  squash over i=1152 per (b, k): block-sum matmul (E4) + free reduces ->
          factor 1/(mod+mod_sq), expanded back via E8 matmul
  routing (u_hat never materialized):
          s[b,od]  = sum_ki u2[ki,b] * (c[i,o]*Wrt[ki,od])   (72 K-chunk matmuls)
          uv[i,o]  = sum_kd Wrt[ki,od] * VU[ki,od],  VU = sum_b u3[b,ki]*v[b,od]
  u2 built via a contiguous DRAM round-trip; u3 = PE-transpose of u2 chunks.
"""
import numpy as np

import concourse.bacc as bacc
import concourse.mybir as mybir
import concourse.tile as tile
from concourse.bass_utils import run_bass_kernel_spmd

F32 = mybir.dt.float32
F32R = mybir.dt.float32r

N_CORES = 8
B_TOT = 180
BPC = 23                     # padded batch per core
SHARD_SIZES = [23, 23, 23, 23, 22, 22, 22, 22]
NHALF = 414                  # caps-conv N split: 18 yx positions x 23 images
ROUTE_ITERS = 3
QK = 72                      # (k,i) contraction chunks: 8*1152/128


def _build_program():
    nc = bacc.Bacc("TRN2", target_bir_lowering=False, debug=False,
                   num_devices=N_CORES)

    # ---------------- I/O ----------------
    x_in = nc.dram_tensor("x_in", [BPC, 784], F32R, kind="ExternalInput")
    w1_in = nc.dram_tensor("w1_in", [81, 256], F32R, kind="ExternalInput")
    b1_in = nc.dram_tensor("b1_in", [256], F32, kind="ExternalInput")
    w2_in = nc.dram_tensor("w2_in", [81, 128, 2, 256], F32R, kind="ExternalInput")
    b2_in = nc.dram_tensor("b2_in", [256], F32, kind="ExternalInput")
    wrt_in = nc.dram_tensor("wrt_in", [9216, 160], F32, kind="ExternalInput")
    e4_in = nc.dram_tensor("e4_in", [128, 4], F32, kind="ExternalInput")
    e8_in = nc.dram_tensor("e8_in", [4, 128], F32, kind="ExternalInput")
    id_in = nc.dram_tensor("id_in", [128, 128], F32, kind="ExternalInput")
    mask_in = nc.dram_tensor("mask_in", [BPC, 1], F32, kind="ExternalInput")
    v_out = nc.dram_tensor("v_out", [BPC, 160], F32, kind="ExternalOutput")

    # DRAM scratch: u in [k, i, b] order (contiguous round-trip)
    u_ram = nc.dram_tensor("u_ram", [8, 1152, BPC], F32)
    cc_in = [nc.dram_tensor(f"cc_in{t}", [128, 90], F32) for t in range(2)]
    cc_out = [nc.dram_tensor(f"cc_out{t}", [128, 90], F32, addr_space="Shared")
              for t in range(2)]

    with tile.TileContext(nc) as tc:
        with tc.tile_pool(name="persist", bufs=1) as pp:

            # ---------- constant / persistent loads ----------
            wrt_sb = pp.tile([128, QK * 160], F32)
            nc.sync.dma_start(
                wrt_sb.rearrange("p (q od) -> p q od", q=QK),
                wrt_in[:, :].rearrange("(q p) od -> p q od", p=128))
            e4_sb = pp.tile([128, 4], F32)
            nc.sync.dma_start(e4_sb, e4_in[:, :])
            e8_sb = pp.tile([4, 128], F32)
            nc.sync.dma_start(e8_sb, e8_in[:, :])
            id_sb = pp.tile([128, 128], F32)
            nc.sync.dma_start(id_sb, id_in[:, :])
            b1_sb = pp.tile([128, 2], F32)
            nc.sync.dma_start(b1_sb, b1_in[:].rearrange("(m p) -> p m", p=128))
            b2_sb = pp.tile([128, 2], F32)
            nc.sync.dma_start(b2_sb, b2_in[:].rearrange("(m p) -> p m", p=128))
            mask_sb = pp.tile([BPC, 1], F32)
            nc.sync.dma_start(mask_sb, mask_in[:, :])

            # ================= conv phase (scoped pools) =================
            with tc.tile_pool(name="conv", bufs=1) as cp, \
                 tc.tile_pool(name="w2p", bufs=6) as w2p, \
                 tc.tile_pool(name="psC", bufs=1, space="PSUM") as psC:

                # ---------- conv1: im2col + matmul ----------
                c1rhs = cp.tile([81, BPC * 400], F32R)
                x3 = x_in[:, :].rearrange("b (y x) -> b y x", y=28)
                for ky in range(9):
                    for kx in range(9):
                        nc.sync.dma_start(
                            c1rhs[9 * ky + kx: 9 * ky + kx + 1, :],
                            x3[:, ky:ky + 20, kx:kx + 20])
                w1_sb = cp.tile([81, 256], F32R)
                nc.sync.dma_start(w1_sb, w1_in[:, :])

                # h layout: [p][c][y 20][par 2][xh 10][b 23] (b innermost,
                # x split even/odd so the caps rhs merges (xh, b) contiguously)
                h_sb = cp.tile([128, 2 * BPC * 400], F32R)
                hv = h_sb.rearrange("p (c y par xh b) -> p c y par xh b",
                                    c=2, y=20, par=2, xh=10)
                for b in range(BPC):
                    for m in range(2):
                        ps = psC.tile([128, 400], F32, tag="c1ps", bufs=2)
                        nc.tensor.matmul(ps, w1_sb[:, 128 * m:128 * (m + 1)],
                                         c1rhs[:, 400 * b:400 * (b + 1)],
                                         start=True, stop=True)
                        nc.scalar.activation(
                            hv[:, m, :, :, :, b],
                            ps.rearrange("p (y xh par) -> p y par xh",
                                         y=20, xh=10),
                            mybir.ActivationFunctionType.Relu,
                            bias=b1_sb[:, m:m + 1])

                # ---------- caps conv ----------
                # psum columns ordered (oy, ox, b); N-halves split on oy
                hv2 = h_sb.rearrange("p (c y par xb) -> p c y par xb",
                                     c=2, y=20, par=2)
                cap_ps = [[psC.tile([128, NHALF], F32, tag=f"cap{m}{j}", bufs=1,
                                    name=f"cap_ps_{m}_{j}")
                           for j in range(2)] for m in range(2)]
                for off in range(81):
                    ky, kx = divmod(off, 9)
                    w2_t = w2p.tile([128, 2 * 256], F32R, tag="w2t")
                    nc.sync.dma_start(
                        w2_t.rearrange("p (c n) -> p c n", c=2),
                        w2_in[off, :, :, :])  # src [p, c, n] contiguous
                    par, xoff = kx % 2, (kx // 2) * BPC
                    for cc in range(2):
                        q = off * 2 + cc
                        # [p][oy 3][(ox b) 138]
                        rhs0 = hv2[:, cc, ky:ky + 5:2, par, xoff:xoff + 138]
                        rhs1 = hv2[:, cc, ky + 6:ky + 11:2, par, xoff:xoff + 138]
                        for m in range(2):
                            lhsT = w2_t[:, cc * 256 + 128 * m: cc * 256 + 128 * (m + 1)]
                            nc.tensor.matmul(cap_ps[m][0], lhsT, rhs0,
                                             start=(q == 0), stop=(q == 161))
                            nc.tensor.matmul(cap_ps[m][1], lhsT, rhs1,
                                             start=(q == 0), stop=(q == 161))

                # evict with bias -> u_b [128, (m, yx, b)]
                u_b = cp.tile([128, 2 * 36 * BPC], F32)
                for m in range(2):
                    for j in range(2):
                        nc.scalar.copy(
                            u_b[:, m * 828 + j * NHALF: m * 828 + (j + 1) * NHALF],
                            cap_ps[m][j])
                    nc.vector.tensor_scalar_add(u_b[:, m * 828:(m + 1) * 828],
                                                u_b[:, m * 828:(m + 1) * 828],
                                                b2_sb[:, m:m + 1])

                # ---------- squash over i per (k, b) ----------
                u_b2 = cp.tile([128, 2 * 36 * BPC], F32)
                nc.vector.tensor_mul(u_b2, u_b, u_b)
                mod_sq = cp.tile([4, 2 * BPC], F32)   # [g][(m, b)]
                part = [cp.tile([4, BPC], F32, tag=f"part{j}", name=f"part_{j}")
                        for j in range(2)]
                for m in range(2):
                    for j in range(2):
                        sq_t = psC.tile([4, 512], F32, tag="sqps", bufs=1,
                                        name=f"sq_t_{m}_{j}")
                        nc.tensor.matmul(
                            sq_t[0:4, 0:NHALF], e4_sb[:, :],
                            u_b2[:, m * 828 + j * NHALF: m * 828 + (j + 1) * NHALF],
                            start=True, stop=True)
                        # cols are (yx 18, b 23); reduce over yx
                        nc.vector.reduce_sum(
                            part[j],
                            sq_t[0:4, 0:NHALF].rearrange(
                                "p (yx b) -> p b yx", yx=18),
                            axis=mybir.AxisListType.X)
                    nc.vector.tensor_add(mod_sq[:, m * BPC:(m + 1) * BPC],
                                         part[0], part[1])
                mod = cp.tile([4, 2 * BPC], F32)
                nc.scalar.sqrt(mod, mod_sq)
                denom = cp.tile([4, 2 * BPC], F32)
                nc.vector.tensor_add(denom, mod, mod_sq)
                fack = cp.tile([4, 2 * BPC], F32)
                nc.vector.reciprocal(fack, denom)
                fac_ps = psC.tile([128, 2 * BPC], F32, tag="facps", bufs=1)
                for m in range(2):
                    nc.tensor.matmul(fac_ps[:, m * BPC:(m + 1) * BPC],
                                     e8_sb[:, :], fack[:, m * BPC:(m + 1) * BPC],
                                     start=True, stop=True)
                u_sq = cp.tile([128, 2 * 36 * BPC], F32)
                for m in range(2):
                    nc.vector.tensor_tensor(
                        u_sq[:, m * 828:(m + 1) * 828].rearrange(
                            "p (yx b) -> p yx b", yx=36),
                        u_b[:, m * 828:(m + 1) * 828].rearrange(
                            "p (yx b) -> p yx b", yx=36),
                        fac_ps[:, m * BPC:(m + 1) * BPC].unsqueeze(1)
                              .broadcast_to((128, 36, BPC)),
                        op=mybir.AluOpType.mult)

                # ---------- u -> DRAM [k, i, b] (fully contiguous) ----------
                for m in range(2):
                    for g in range(4):
                        k = 4 * m + g
                        nc.sync.dma_start(
                            u_ram[k, :, :],
                            u_sq[32 * g:32 * (g + 1), m * 828:(m + 1) * 828])
            # ============== end conv phase (pools freed) ==============

            with tc.tile_pool(name="routing", bufs=1) as rp, \
                 tc.tile_pool(name="psR", bufs=2, space="PSUM") as psR:
                u2_sb = rp.tile([128, QK * BPC], F32)   # [p][(k, ic)][b]
                for k in range(8):
                    nc.sync.dma_start(
                        u2_sb[:, k * 9 * BPC:(k + 1) * 9 * BPC].rearrange(
                            "p (ic b) -> p ic b", ic=9),
                        u_ram[k, :, :].rearrange("(ic p) b -> p ic b", p=128))
                # u3 = PE-transpose of u2 chunks
                u3_sb = rp.tile([BPC, 9216], F32)
                for q in range(QK):
                    tp = psR.tile([32, 128], F32, tag="tps", bufs=2)
                    nc.tensor.transpose(tp[0:BPC, :],
                                        u2_sb[:, q * BPC:(q + 1) * BPC],
                                        id_sb)
                    nc.scalar.copy(u3_sb[:, q * 128:(q + 1) * 128], tp[0:BPC, :])

                # ---------- routing ----------
                cw_sb = rp.tile([128, QK * 160], F32)
                b_ij = [rp.tile([128, 90], F32, tag=f"bij{t}", name=f"b_ij_{t}")
                        for t in range(2)]
                c_sb = rp.tile([128, 90], F32)
                uvp = rp.tile([128, QK * 10], F32)   # [p][(ic, k)][o]
                uv9 = rp.tile([128, 90], F32)
                uvr = [rp.tile([128, 90], F32, tag=f"uvr{t}", name=f"uvr_{t}")
                       for t in range(2)]

                v3 = rp.tile([BPC, 160], F32)
                v3m = rp.tile([BPC, 160], F32)
                s2 = rp.tile([BPC, 160], F32)
                msq = rp.tile([BPC, 16], F32)
                mroot = rp.tile([BPC, 16], F32)
                sden = rp.tile([BPC, 16], F32)
                fac = rp.tile([BPC, 16], F32)
                fac2 = rp.tile([BPC, 16], F32)
                smax = rp.tile([128, 9], F32)
                ssum = rp.tile([128, 9], F32)
                srec = rp.tile([128, 9], F32)
                sexp = rp.tile([128, 90], F32)

                for it in range(ROUTE_ITERS):
                    # --- c_ij ---
                    if it > 0:
                        bij = b_ij[it - 1]
                        b3 = bij.rearrange("p (ic o) -> p ic o", ic=9)
                        nc.vector.reduce_max(smax, b3, axis=mybir.AxisListType.X)
                        nc.vector.tensor_tensor(
                            sexp.rearrange("p (ic o) -> p ic o", ic=9), b3,
                            smax.unsqueeze(2).broadcast_to((128, 9, 10)),
                            op=mybir.AluOpType.subtract)
                        nc.scalar.activation(sexp, sexp,
                                             mybir.ActivationFunctionType.Exp)
                        nc.vector.reduce_sum(
                            ssum, sexp.rearrange("p (ic o) -> p ic o", ic=9),
                            axis=mybir.AxisListType.X)
                        nc.vector.reciprocal(srec, ssum)
                        nc.vector.tensor_tensor(
                            c_sb.rearrange("p (ic o) -> p ic o", ic=9),
                            sexp.rearrange("p (ic o) -> p ic o", ic=9),
                            srec.unsqueeze(2).broadcast_to((128, 9, 10)),
                            op=mybir.AluOpType.mult)
                        # --- CW = c * Wrt ---
                        for q in range(QK):
                            ic = q % 9
                            eng = nc.vector if q % 3 else nc.gpsimd
                            eng.tensor_tensor(
                                cw_sb[:, q * 160:(q + 1) * 160].rearrange(
                                    "p (o d) -> p o d", o=10),
                                wrt_sb[:, q * 160:(q + 1) * 160].rearrange(
                                    "p (o d) -> p o d", o=10),
                                c_sb[:, ic * 10:(ic + 1) * 10].unsqueeze(2)
                                    .broadcast_to((128, 10, 16)),
                                op=mybir.AluOpType.mult)
                        rhs_src = cw_sb
                    else:
                        rhs_src = wrt_sb

                    # --- s = sum_q u2_q^T @ rhs_q ---
                    s_ps = psR.tile([BPC, 160], F32, tag="sps", bufs=2)
                    for q in range(QK):
                        nc.tensor.matmul(s_ps, u2_sb[:, q * BPC:(q + 1) * BPC],
                                         rhs_src[:, q * 160:(q + 1) * 160],
                                         start=(q == 0), stop=(q == QK - 1))

                    # --- v = squash(s, over o) ---
                    scale = 0.1 if it == 0 else 1.0
                    nc.scalar.activation(s2, s_ps,
                                         mybir.ActivationFunctionType.Square,
                                         scale=scale)
                    nc.vector.reduce_sum(
                        msq, s2.rearrange("p (o d) -> p d o", o=10),
                        axis=mybir.AxisListType.X)
                    nc.scalar.sqrt(mroot, msq)
                    nc.vector.tensor_add(sden, mroot, msq)
                    nc.vector.reciprocal(fac, sden)
                    if it == 0:
                        nc.vector.tensor_scalar_mul(fac2, fac, 0.1)
                        facv = fac2
                    else:
                        facv = fac
                    nc.vector.tensor_tensor(
                        v3.rearrange("p (o d) -> p o d", o=10),
                        s_ps.rearrange("p (o d) -> p o d", o=10),
                        facv.unsqueeze(1).broadcast_to((BPC, 10, 16)),
                        op=mybir.AluOpType.mult)

                    if it == ROUTE_ITERS - 1:
                        nc.sync.dma_start(v_out[:, :], v3)
                        break

                    nc.vector.tensor_scalar_mul(v3m, v3, mask_sb[:, 0:1])

                    # --- VU_q = u3_q^T @ v3m ; uv = sum_kd Wrt .* VU ---
                    for q in range(QK):
                        k, ic = divmod(q, 9)
                        vu_ps = psR.tile([128, 160], F32, tag="vups", bufs=2)
                        nc.tensor.matmul(vu_ps, u3_sb[:, q * 128:(q + 1) * 128],
                                         v3m, start=True, stop=True)
                        tmp = rp.tile([128, 160], F32, tag="vutmp", bufs=4)
                        nc.vector.tensor_mul(tmp, vu_ps,
                                             wrt_sb[:, q * 160:(q + 1) * 160])
                        nc.vector.reduce_sum(
                            uvp[:, (ic * 8 + k) * 10:(ic * 8 + k + 1) * 10],
                            tmp.rearrange("p (o d) -> p o d", o=10),
                            axis=mybir.AxisListType.X)
                    # sum over k: view [p][ic][o][k] reduce X
                    nc.vector.reduce_sum(
                        uv9.rearrange("p (ic o) -> p ic o", ic=9),
                        uvp.rearrange("p (ic k o) -> p ic o k", ic=9, k=8),
                        axis=mybir.AxisListType.X)

                    # --- AllReduce + b_ij update ---
                    nc.sync.dma_start(cc_in[it][:, :], uv9)
                    nc.gpsimd.collective_compute(
                        "AllReduce", mybir.AluOpType.add,
                        replica_groups=[list(range(N_CORES))],
                        ins=[cc_in[it][:, :].opt()],
                        outs=[cc_out[it][:, :].opt()])
                    nc.sync.dma_start(uvr[it], cc_out[it][:, :])
                    if it == 0:
                        nc.vector.tensor_scalar_mul(b_ij[0], uvr[0],
                                                    1.0 / B_TOT)
                    else:
                        nc.vector.scalar_tensor_tensor(
                            b_ij[it], uvr[it], 1.0 / B_TOT, b_ij[it - 1],
                            op0=mybir.AluOpType.mult, op1=mybir.AluOpType.add)

    nc.compile()
    return nc


_CACHE = {}


def _get_program():
    if "nc" not in _CACHE:
        _CACHE["nc"] = _build_program()
    return _CACHE["nc"]


def _prep_inputs(x, conv1_w, conv1_b, caps_w, caps_b, W_route):
    x = np.asarray(x, np.float32).reshape(B_TOT, 784)
    w1 = np.ascontiguousarray(
        np.asarray(conv1_w, np.float32).reshape(256, 81).T)
    b1 = np.asarray(conv1_b, np.float32)
    w2 = np.ascontiguousarray(
        np.asarray(caps_w, np.float32).reshape(256, 256, 81)
        .transpose(2, 1, 0).reshape(81, 2, 128, 256).transpose(0, 2, 1, 3))
    b2 = np.asarray(caps_b, np.float32).reshape(256)
    wrt = np.ascontiguousarray(
        np.asarray(W_route, np.float32)[0].transpose(3, 0, 1, 2)
    ).reshape(9216, 160)

    e4 = np.zeros((128, 4), np.float32)
    for p in range(128):
        e4[p, p // 32] = 1.0
    e8 = np.zeros((4, 128), np.float32)
    for p in range(128):
        e8[p // 32, p] = 1.0
    ident = np.eye(128, dtype=np.float32)

    in_maps = []
    off = 0
    for c in range(N_CORES):
        nb = SHARD_SIZES[c]
        xs = x[off:off + nb]
        off += nb
        if nb < BPC:
            xs = np.concatenate([xs, np.repeat(xs[:1], BPC - nb, 0)], 0)
        mask = np.zeros((BPC, 1), np.float32)
        mask[:nb] = 1.0
        in_maps.append({
            "x_in": np.ascontiguousarray(xs),
            "w1_in": w1, "b1_in": b1, "w2_in": w2, "b2_in": b2,
            "wrt_in": wrt, "e4_in": e4, "e8_in": e8, "id_in": ident,
            "mask_in": mask,
        })
    return in_maps


def kernel(x, conv1_w, conv1_b, caps_w, caps_b, W_route):
    nc = _get_program()
    in_maps = _prep_inputs(x, conv1_w, conv1_b, caps_w, caps_b, W_route)
    res = run_bass_kernel_spmd(nc, in_maps, core_ids=list(range(N_CORES)))
    outs = []
    for c in range(N_CORES):
        outs.append(res.results[c]["v_out"][:SHARD_SIZES[c]])
    v = np.concatenate(outs, 0).reshape(B_TOT, 10, 16, 1)
    return v.astype(np.float32)
